# revision 1
# baseline (speedup 1.0000x reference)
"""Trainium2 Bass kernel for nn_DecoderLayer_66408784331382.

Single transformer decoder layer (RMSNorm + GQA attention w/ RoPE + RMSNorm +
SwiGLU MLP), tensor-parallel over 8 NeuronCores:

  - per core: 4 of 32 Q heads, 1 of 8 KV heads, 1024 of 8192 MLP inter cols,
    matching row-shards of wo / w_down.
  - all on-device activations are kept transposed ([hid, tok] etc.) so that
    every matmul is transpose-free; the host supplies hidden_states.T.
  - RMS statistics use an ACT Square pass + ones-column matmul (partition
    reduction); softmax denominators come from a ones-column appended to V in
    the PV matmul; per-token scaling uses partition-stride-0 broadcast DMAs.
  - one on-device fp32 AllReduce joins attention output partials before the
    second RMSNorm; the final down-proj partials (+ x1/8 each) are summed on
    the host during unsharding.
  - attention path is float32r (full-rate fp32 matmuls); the MLP runs bf16.

kernel(**inputs) takes the FULL fp32 inputs of reference.setup_inputs() and
returns the FULL [1, 2048, 2048] fp32 output.
"""

import sys

if "/opt/trn_rl_repo" not in sys.path:
    sys.path.insert(0, "/opt/trn_rl_repo")

import numpy as np
import ml_dtypes

import concourse.bass as bass
import concourse.mybir as mybir
import concourse.tile as tile
from concourse import bacc
from concourse.bass_utils import run_bass_kernel_spmd

# ---- problem constants (hardcoded per contract) ----
N_CORES = 8
S = 2048
HID = 2048
HD = 64
NH = 32
INTER = 8192
EPS = 1e-6

QD = (NH // N_CORES) * HD        # 256 local q cols
INTER_LOC = INTER // N_CORES     # 1024
SCALE = 1.0 / np.sqrt(HD)

F32 = mybir.dt.float32
F32R = mybir.dt.float32r
BF16 = mybir.dt.bfloat16

P = 128
Q = 512      # phase-1 token quarter
C = 1024     # phase-4 token chunk
ARDT = mybir.dt.float16  # collective dtype
AF = mybir.ActivationFunctionType
ALU = mybir.AluOpType


def _bcast(ap, parts):
    """View a [1, N] AP as [parts, N] via partition-stride-0 (DMA broadcast)."""
    return bass.AP(tensor=ap.tensor, offset=ap.offset,
                   ap=[[0, parts]] + [list(p) for p in ap.ap[1:]])


def build():
    nc = bacc.Bacc("TRN2", target_bir_lowering=False, debug=False,
                   num_devices=N_CORES)

    hT_d = nc.dram_tensor("hT", [HID, S], F32R, kind="ExternalInput")
    sin4_d = nc.dram_tensor("sin4", [P, S], F32R, kind="ExternalInput")
    cos4_d = nc.dram_tensor("cos4", [P, S], F32R, kind="ExternalInput")
    wq_d = nc.dram_tensor("wq", [HID, QD], F32R, kind="ExternalInput")
    wkv_d = nc.dram_tensor("wkv", [HID, 2 * HD], F32R, kind="ExternalInput")
    wo_d = nc.dram_tensor("wo", [QD, HID], F32R, kind="ExternalInput")
    wg_d = nc.dram_tensor("wg", [HID, INTER_LOC], BF16, kind="ExternalInput")
    wu_d = nc.dram_tensor("wu", [HID, INTER_LOC], BF16, kind="ExternalInput")
    wd_d = nc.dram_tensor("wd", [INTER_LOC, HID], BF16, kind="ExternalInput")
    ident_d = nc.dram_tensor("ident", [P, P], F32R, kind="ExternalInput")
    masks_d = nc.dram_tensor("masks", [P, 4 * 512], F32R, kind="ExternalInput")
    outT_d = nc.dram_tensor("outT", [HID, S], F32, kind="ExternalOutput")

    with tile.TileContext(nc) as tc, nc.allow_low_precision(
            reason="float32r is fp32 bits; reciprocal outputs are fp32-width"):
        with (
            tc.tile_pool(name="const", bufs=1) as const,
            tc.tile_pool(name="dramp", bufs=1, space="DRAM") as dram,
        ):
            ones1 = const.tile([P, 1], F32R)
            eps1 = const.tile([P, 1], F32)
            nc.gpsimd.memset(eps1, EPS)
            # f32r memset fails the walrus ISA check; masks[:,0,511] is all-1.0
            nc.sync.dma_start(
                ones1, bass.AP(tensor=masks_d.tensor
                               if hasattr(masks_d, "tensor") else masks_d,
                               offset=511, ap=[[4 * 512, P], [0, 1]]))

            ar_in = [dram.tile([HID, C], ARDT, name=f"ar_in{i}",
                               tag=f"ar_in{i}") for i in range(2)]
            ar_out = [dram.tile([HID, C], ARDT, addr_space="Shared",
                                name=f"ar_out{i}", tag=f"ar_out{i}")
                      for i in range(2)]
            bc1_dram = dram.tile([4, Q], F32R)
            bc2_dram = dram.tile([2, 8, 512], F32R)
            bc4_dram = dram.tile([2, C], F32R)

            # ======== attention scope (phases 1-3 share these tensors) ======
            with tc.tile_pool(name="keep", bufs=1) as keep:
                sin4 = keep.tile([P, S], F32R)
                cos4 = keep.tile([P, S], F32R)
                ident = keep.tile([P, P], F32R)
                masks = keep.tile([P, 4, 512], F32R)
                nc.sync.dma_start(sin4, sin4_d[:, :])
                nc.sync.dma_start(cos4, cos4_d[:, :])
                nc.sync.dma_start(ident, ident_d[:, :])
                nc.sync.dma_start(
                    masks, masks_d[:, :].rearrange("p (t n) -> p t n", t=4))
                qT = [keep.tile([P, S], F32R, tag=f"qT{m}", name=f"qT{m}") for m in range(2)]
                kTdup = keep.tile([P, S], F32R, tag="kTdup")
                v_ones = keep.tile([P, 16, HD + 1], F32R, tag="v_ones")
                attnT = [keep.tile([P, S], F32R, tag=f"attnT{m}", name=f"attnT{m}")
                         for m in range(2)]
                nc.sync.dma_start(
                    v_ones[:, :, HD:HD + 1],
                    bass.AP(tensor=masks_d.tensor
                            if hasattr(masks_d, "tensor") else masks_d,
                            offset=511, ap=[[4 * 512, P], [0, 16], [0, 1]]))

                # ---- Phase 1: RMS1 + QKV + RoPE, per 512-token quarter ----
                with (
                    tc.tile_pool(name="p1w", bufs=1) as p1w,
                    tc.tile_pool(name="p1x", bufs=1) as p1x,
                    tc.tile_pool(name="p1s", bufs=1) as p1s,
                    tc.tile_pool(name="p1ps", bufs=2, space="PSUM") as p1ps,
                    tc.tile_pool(name="p1ps_s", bufs=1, space="PSUM") as p1pss,
                ):
                    wq_all = p1w.tile([P, 16, QD], F32R)
                    wkv_all = p1w.tile([P, 16, 2 * HD], F32R)
                    nc.scalar.dma_start(
                        wq_all, wq_d[:, :].rearrange("(t p) m -> p t m", p=P))
                    nc.scalar.dma_start(
                        wkv_all, wkv_d[:, :].rearrange("(t p) m -> p t m", p=P))
                    xn1 = p1x.tile([P, 16, Q], F32R, tag="xn1")

                    for q4 in range(4):
                        qc = slice(Q * q4, Q * (q4 + 1))
                        # RMS1 stats
                        ssq = p1pss.tile([1, Q], F32, tag="ssq")
                        for t4 in range(4):
                            nc.sync.dma_start(
                                xn1[:, 4 * t4:4 * (t4 + 1), :],
                                hT_d[512 * t4:512 * (t4 + 1), qc].rearrange(
                                    "(t p) m -> p t m", p=P))
                        for kt in range(16):
                            xt = xn1[:, kt, :]
                            sq = p1s.tile([P, Q], F32R, tag="sq", bufs=3)
                            nc.scalar.activation(sq, xt, AF.Square)
                            nc.tensor.matmul(ssq, ones1, sq,
                                             start=(kt == 0), stop=(kt == 15))
                        rms = p1s.tile([1, Q], F32R, tag="rms", bufs=2)
                        nc.scalar.activation(rms, ssq, AF.Sqrt,
                                             bias=eps1[0:1, :], scale=1.0 / HID)
                        inv = p1s.tile([1, Q], F32R, tag="inv", bufs=2)
                        nc.vector.reciprocal(inv, rms)
                        invb = p1s.tile([P, Q], F32R, tag="invb", bufs=2)
                        nc.sync.dma_start(bc1_dram[q4:q4 + 1, :], inv)
                        nc.sync.dma_start(invb, _bcast(bc1_dram[q4:q4 + 1, :], P))
                        for kt in range(16):
                            nc.vector.tensor_mul(xn1[:, kt, :],
                                                 xn1[:, kt, :], invb)

                        # QKV projections (transposed outputs)
                        q_ps = [p1ps.tile([P, Q], F32, tag=f"qps{m}", name=f"qps{m}")
                                for m in range(2)]
                        kv_ps = p1ps.tile([P, Q], F32, tag="kvps")
                        for kt in range(16):
                            st, sp = (kt == 0), (kt == 15)
                            for m in range(2):
                                nc.tensor.matmul(
                                    q_ps[m], wq_all[:, kt, P * m:P * (m + 1)],
                                    xn1[:, kt, :], start=st, stop=sp)
                            nc.tensor.matmul(kv_ps, wkv_all[:, kt, :],
                                             xn1[:, kt, :], start=st, stop=sp)

                        # RoPE eviction (sin4 rows carry the rotate-half
                        # sign: +sinT for x0, -sinT for x1 source rows):
                        # out = ps*cos + swap_half(ps)*sinA
                        for m in range(2):
                            s1 = p1s.tile([P, Q], F32R, tag="s1", bufs=2)
                            s2 = p1s.tile([P, Q], F32R, tag="s2", bufs=2)
                            nc.vector.tensor_mul(s1, q_ps[m], cos4[:, qc])
                            for b in range(2):
                                x0 = slice(64 * b, 64 * b + 32)
                                x1s = slice(64 * b + 32, 64 * b + 64)
                                nc.vector.tensor_mul(
                                    s2[x0, :], q_ps[m][x1s, :], sin4[x1s, qc])
                                nc.vector.tensor_mul(
                                    s2[x1s, :], q_ps[m][x0, :], sin4[x0, qc])
                            nc.vector.tensor_add(qT[m][:, qc], s1, s2)
                        # RoPE eviction: k, duplicated into rows 64:128
                        s1 = p1s.tile([64, Q], F32R, tag="s1k", bufs=2)
                        s2 = p1s.tile([64, Q], F32R, tag="s2k", bufs=2)
                        nc.vector.tensor_mul(s1, kv_ps[0:64, :], cos4[0:64, qc])
                        nc.vector.tensor_mul(
                            s2[0:32, :], kv_ps[32:64, :], sin4[32:64, qc])
                        nc.vector.tensor_mul(
                            s2[32:64, :], kv_ps[0:32, :], sin4[0:32, qc])
                        nc.vector.tensor_add(kTdup[0:64, qc], s1, s2)
                        nc.vector.tensor_copy(kTdup[64:128, qc], kTdup[0:64, qc])
                        # v: vT then PE-transpose into v_ones
                        vt = p1s.tile([64, Q], F32R, tag="vt", bufs=2)
                        nc.vector.tensor_copy(vt, kv_ps[64:128, :])
                        for j in range(4):
                            ktg = 4 * q4 + j
                            vtp = p1pss.tile([P, HD], F32R, tag="vtp")
                            nc.tensor.transpose(
                                vtp, vt[:, P * j:P * (j + 1)],
                                ident[0:64, 0:64])
                            nc.vector.tensor_copy(v_ones[:, ktg, 0:HD], vtp)

                # ---- Phases 2+3 interleaved per token half: attention
                #      for half h, o-proj for half h, AllReduce(h).  The
                #      attention work of half 1 overlaps AllReduce(0). ----
                with (
                    tc.tile_pool(name="p2pr", bufs=3) as p2pr,
                    tc.tile_pool(name="p2sm", bufs=2) as p2sm,
                    tc.tile_pool(name="p3w", bufs=1) as p3w,
                    tc.tile_pool(name="p3o", bufs=3) as p3o,
                    tc.tile_pool(name="p2ps", bufs=2, space="PSUM") as p2ps,
                    tc.tile_pool(name="p2pv", bufs=1, space="PSUM") as p2pv,
                    tc.tile_pool(name="p3ps", bufs=2, space="PSUM") as p3ps,
                ):
                    wo_all = p3w.tile([P, 2, HID], F32R)
                    nc.scalar.dma_start(
                        wo_all, wo_d[:, :].rearrange("(t p) m -> p t m", p=P))
                    for c3 in range(2):
                        for qc4 in range(2 * c3, 2 * c3 + 2):
                            for m in range(2):
                                qs = slice(512 * qc4, 512 * (qc4 + 1))
                                pv = [p2pv.tile([HD + 1, 512], F32,
                                                tag=f"pv{b}", name=f"pv{b}")
                                      for b in range(2)]
                                nkt = 4 * qc4 + 4
                                for kt in range(nkt):
                                    st, sp = (kt == 0), (kt == nkt - 1)
                                    for b in range(2):
                                        rows = slice(64 * b, 64 * (b + 1))
                                        sc = p2ps.tile([P, 512], F32,
                                                       tag=f"sc{b}")
                                        nc.tensor.matmul(
                                            sc,
                                            kTdup[rows, P * kt:P * (kt + 1)],
                                            qT[m][rows, qs],
                                            start=True, stop=True)
                                        pr = p2pr.tile([P, 512], F32R,
                                                       tag=f"pr{b}")
                                        nc.scalar.activation(
                                            pr, sc, AF.Exp, scale=float(SCALE))
                                        if kt >= 4 * qc4:
                                            nc.vector.tensor_mul(
                                                pr, pr,
                                                masks[:, kt - 4 * qc4, :])
                                        nc.tensor.matmul(
                                            pv[b], v_ones[:, kt, :], pr,
                                            start=st, stop=sp)
                                for b in range(2):
                                    rec = p2sm.tile([1, 512], F32R,
                                                    tag=f"rec{b}")
                                    nc.vector.reciprocal(
                                        rec, pv[b][HD:HD + 1, :])
                                    slot = bc2_dram[b:b + 1, 4 * m + qc4, :]
                                    nc.sync.dma_start(slot, rec)
                                    recb = p2sm.tile([64, 512], F32R,
                                                     tag=f"recb{b}")
                                    nc.sync.dma_start(recb, _bcast(slot, 64))
                                    nc.vector.tensor_mul(
                                        attnT[m][64 * b:64 * (b + 1), qs],
                                        pv[b][0:HD, :], recb)
                        # o-proj for this half -> ar_in[c3]
                        for hm in range(16):
                            osb = p3o.tile([P, C], ARDT, tag="osb")
                            for nq in range(2):
                                qc4 = 2 * c3 + nq
                                qs = slice(512 * qc4, 512 * (qc4 + 1))
                                ops = p3ps.tile([P, 512], F32, tag="ops")
                                for kt2 in range(2):
                                    nc.tensor.matmul(
                                        ops,
                                        wo_all[:, kt2, P * hm:P * (hm + 1)],
                                        attnT[kt2][:, qs],
                                        start=(kt2 == 0), stop=(kt2 == 1))
                                nc.scalar.copy(
                                    osb[:, 512 * nq:512 * (nq + 1)], ops)
                            nc.gpsimd.dma_start(
                                ar_in[c3][P * hm:P * (hm + 1), :], osb)
                        # AllReduce for this token half (overlaps what follows)
                        nc.gpsimd.collective_compute(
                            "AllReduce", ALU.add,
                            replica_groups=[list(range(N_CORES))],
                            ins=[ar_in[c3][:, :].opt()],
                            outs=[ar_out[c3][:, :].opt()])

            # ---- Phase 4: x1 + RMS2 + SwiGLU MLP, per 1024-token chunk ----
            with (
                tc.tile_pool(name="p4x", bufs=1) as p4x,
                tc.tile_pool(name="p4s", bufs=1) as p4s,
                tc.tile_pool(name="p4w", bufs=1) as p4w,
                tc.tile_pool(name="p4ps_s", bufs=1, space="PSUM") as p4pss,
                tc.tile_pool(name="p4ps_gu", bufs=1, space="PSUM") as p4gu,
                tc.tile_pool(name="p4ps_d", bufs=1, space="PSUM") as p4d,
            ):
                x1 = p4x.tile([P, 16, C], F32R, tag="x1")
                xn2 = p4x.tile([P, 16, C], BF16, tag="xn2")
                hmlp = p4x.tile([P, 8, C], BF16, tag="hmlp")
                for c2 in range(2):
                    cc = slice(C * c2, C * (c2 + 1))
                    # x1 = hidden + attn_out ; RMS2 stats
                    ssq2 = p4pss.tile([1, C], F32, tag="ssq2")
                    for kt in range(16):
                        rs = slice(P * kt, P * (kt + 1))
                        th = p4s.tile([P, C], F32R, tag="th", bufs=2)
                        ta = p4s.tile([P, C], ARDT, tag="ta", bufs=2)
                        nc.sync.dma_start(th, hT_d[rs, cc])
                        nc.sync.dma_start(ta, ar_out[c2][rs, :])
                        nc.vector.tensor_add(x1[:, kt, :], th, ta)
                        sq = p4s.tile([P, C], F32R, tag="sq2", bufs=2)
                        nc.scalar.activation(sq, x1[:, kt, :], AF.Square)
                        for n in range(2):
                            nc.tensor.matmul(
                                ssq2[:, 512 * n:512 * (n + 1)], ones1,
                                sq[:, 512 * n:512 * (n + 1)],
                                start=(kt == 0), stop=(kt == 15))
                    rms = p4s.tile([1, C], F32R, tag="rms2", bufs=2)
                    nc.scalar.activation(rms, ssq2, AF.Sqrt,
                                         bias=eps1[0:1, :], scale=1.0 / HID)
                    inv = p4s.tile([1, C], F32R, tag="inv2", bufs=2)
                    nc.vector.reciprocal(inv, rms)
                    invb = p4s.tile([P, C], F32R, tag="invb2", bufs=1)
                    nc.sync.dma_start(bc4_dram[c2:c2 + 1, :], inv)
                    nc.sync.dma_start(invb, _bcast(bc4_dram[c2:c2 + 1, :], P))
                    for kt in range(16):
                        nc.vector.tensor_mul(xn2[:, kt, :], x1[:, kt, :], invb)

                    # gate/up + silu*up (bf16)
                    for iq in range(8):
                        gps = p4gu.tile([P, C], F32, tag="g")
                        ups = p4gu.tile([P, C], F32, tag="u")
                        wg_t = p4w.tile([P, 16, P], BF16, tag="wgt", bufs=2)
                        wu_t = p4w.tile([P, 16, P], BF16, tag="wut", bufs=2)
                        nc.scalar.dma_start(
                            wg_t, wg_d[:, P * iq:P * (iq + 1)].rearrange(
                                "(t p) m -> p t m", p=P))
                        nc.scalar.dma_start(
                            wu_t, wu_d[:, P * iq:P * (iq + 1)].rearrange(
                                "(t p) m -> p t m", p=P))
                        for kt in range(16):
                            st, sp = (kt == 0), (kt == 15)
                            for n in range(2):
                                ns = slice(512 * n, 512 * (n + 1))
                                nc.tensor.matmul(gps[:, ns], wg_t[:, kt, :],
                                                 xn2[:, kt, ns],
                                                 start=st, stop=sp)
                                nc.tensor.matmul(ups[:, ns], wu_t[:, kt, :],
                                                 xn2[:, kt, ns],
                                                 start=st, stop=sp)
                        sg = p4s.tile([P, C], BF16, tag="sg", bufs=2)
                        nc.scalar.activation(sg, gps, AF.Silu)
                        nc.vector.tensor_mul(hmlp[:, iq, :], sg, ups)

                    # down proj + (x1/8) residual share -> outT
                    for hm in range(16):
                        dps = p4d.tile([P, C], F32, tag="d")
                        wd_t = p4w.tile([P, 8, P], BF16, tag="wdt", bufs=2)
                        nc.scalar.dma_start(
                            wd_t, wd_d[:, P * hm:P * (hm + 1)].rearrange(
                                "(t p) m -> p t m", p=P))
                        for kt8 in range(8):
                            for n in range(2):
                                ns = slice(512 * n, 512 * (n + 1))
                                nc.tensor.matmul(dps[:, ns], wd_t[:, kt8, :],
                                                 hmlp[:, kt8, ns],
                                                 start=(kt8 == 0),
                                                 stop=(kt8 == 7))
                        dsb = p4s.tile([P, C], F32, tag="dsb", bufs=2)
                        nc.vector.scalar_tensor_tensor(
                            dsb, x1[:, hm, :], 1.0 / N_CORES, dps,
                            op0=ALU.mult, op1=ALU.add)
                        nc.gpsimd.dma_start(outT_d[P * hm:P * (hm + 1), cc], dsb)

    nc.compile()
    return nc


_CACHE = {}


def _get_nc():
    if "nc" not in _CACHE:
        _CACHE["nc"] = build()
    return _CACHE["nc"]


def _prep_inputs(inputs):
    """Shard + preprocess full inputs into 8 per-core in_maps."""
    f = lambda k: np.asarray(inputs[k], dtype=np.float32)
    hidden = f("hidden_states")[0]                 # [S, HID]
    sin_t, cos_t = f("sin_table"), f("cos_table")  # [S, 32]
    ln1, ln2 = f("ln1_w"), f("ln2_w")
    wq = f("wq") * ln1[:, None]
    wk = f("wk") * ln1[:, None]
    wv = f("wv") * ln1[:, None]
    wo = f("wo")
    wg = (f("w_gate") * ln2[:, None]).astype(ml_dtypes.bfloat16)
    wu = (f("w_up") * ln2[:, None]).astype(ml_dtypes.bfloat16)
    wd = f("w_down").astype(ml_dtypes.bfloat16)

    hT = np.ascontiguousarray(hidden.T)
    # rows per 64-block: [+sinT (x0 source); -sinT (x1 source)]
    sin4 = np.ascontiguousarray(
        np.tile(np.concatenate([sin_t.T, -sin_t.T], axis=0), (2, 1)))
    cos4 = np.ascontiguousarray(np.tile(cos_t.T, (4, 1)))
    ident = np.eye(P, dtype=np.float32)
    rr = np.arange(P)[:, None]
    cols = np.arange(512)[None, :]
    masks = np.concatenate(
        [(rr + 128 * t <= cols).astype(np.float32) for t in range(4)],
        axis=1)

    in_maps = []
    for c in range(N_CORES):
        qs = slice(QD * c, QD * (c + 1))
        ks = slice(HD * c, HD * (c + 1))
        isl = slice(INTER_LOC * c, INTER_LOC * (c + 1))
        in_maps.append({
            "hT": hT,
            "sin4": sin4,
            "cos4": cos4,
            "wq": np.ascontiguousarray(wq[:, qs]),
            "wkv": np.ascontiguousarray(
                np.concatenate([wk[:, ks], wv[:, ks]], axis=1)),
            "wo": np.ascontiguousarray(wo[qs, :]),
            "wg": np.ascontiguousarray(wg[:, isl]),
            "wu": np.ascontiguousarray(wu[:, isl]),
            "wd": np.ascontiguousarray(wd[isl, :]),
            "ident": ident,
            "masks": masks,
        })
    return in_maps


def kernel(**inputs):
    nc = _get_nc()
    in_maps = _prep_inputs(inputs)
    res = run_bass_kernel_spmd(nc, in_maps, core_ids=list(range(N_CORES)))
    acc = np.zeros((HID, S), dtype=np.float32)
    for c in range(N_CORES):
        acc += res.results[c]["outT"]
    return np.ascontiguousarray(acc.T)[None, :, :]



# revision 6
# speedup vs baseline: 1.1103x; 1.1103x over previous
"""Trainium2 Bass kernel for nn_DecoderLayer_66408784331382 (v2).

Single transformer decoder layer (RMSNorm + GQA attention w/ RoPE + RMSNorm +
SwiGLU MLP), tensor-parallel over 8 NeuronCores:

  - per core: 4 of 32 Q heads, 1 of 8 KV heads, 1024 of 8192 MLP inter cols,
    matching row-shards of wo / w_down.
  - activations transposed on device ([hid, tok]) so every matmul is
    transpose-free; the host supplies hidden_states.T (fp32 + bf16 copies).
  - RMS1 is folded *after* the QKV projection: (x*inv)@W == (x@W)*inv, so the
    QKV matmuls run immediately on raw x while the sum-of-squares statistics
    (ACT Square + ones-column matmul) compute concurrently; the per-token
    inv_rms is folded into the RoPE cos/sin multipliers at PSUM eviction.
  - all reciprocals run on [128,N] broadcast tiles (128 DVE lanes) instead of
    [1,N] rows; broadcast via DRAM round-trip with partition-stride-0 APs.
  - attention math (scores, exp, PV, o-proj) and the MLP run in bf16 on the
    PE (same rate as fp32r but half the DVE/ACT/DMA cost); accumulation and
    softmax statistics stay fp32 in PSUM.
  - the attention-output AllReduce is split into four 512-token fp16 chunks;
    MLP work for chunk c is emitted interleaved with attention of later
    quarters so every collective hides behind PE work.
  - down-proj partials (+ x1/8 residual share) are written as fp16 and summed
    on the host during unsharding.

kernel(**inputs) takes the FULL fp32 inputs of reference.setup_inputs() and
returns the FULL [1, 2048, 2048] fp32 output.
"""

import sys

if "/opt/trn_rl_repo" not in sys.path:
    sys.path.insert(0, "/opt/trn_rl_repo")

import numpy as np
import ml_dtypes

import concourse.bass as bass
import concourse.mybir as mybir
import concourse.tile as tile
from concourse import bacc
from concourse.bass_utils import run_bass_kernel_spmd

# ---- problem constants (hardcoded per contract) ----
N_CORES = 8
S = 2048
HID = 2048
HD = 64
NH = 32
INTER = 8192
EPS = 1e-6

QD = (NH // N_CORES) * HD        # 256 local q cols (2 tiles of 128)
INTER_LOC = INTER // N_CORES     # 1024
SCALE = 1.0 / np.sqrt(HD)

F32 = mybir.dt.float32
F32R = mybir.dt.float32r
BF16 = mybir.dt.bfloat16
F16 = mybir.dt.float16

P = 128
XC = 256     # phase-1 token chunk
Q = 512      # attention / MLP token quarter
NQ = S // Q  # 4
ARDT = F16   # collective dtype
AF = mybir.ActivationFunctionType
ALU = mybir.AluOpType


def _bcast(ap, parts):
    """View a [1, N] AP as [parts, N] via partition-stride-0 (DMA broadcast)."""
    return bass.AP(tensor=ap.tensor, offset=ap.offset,
                   ap=[[0, parts]] + [list(p) for p in ap.ap[1:]])


def build():
    nc = bacc.Bacc("TRN2", target_bir_lowering=False, debug=False,
                   num_devices=N_CORES)

    hT_d = nc.dram_tensor("hT", [HID, S], F32R, kind="ExternalInput")
    hTb_d = nc.dram_tensor("hTb", [HID, S], BF16, kind="ExternalInput")
    sin4_d = nc.dram_tensor("sin4", [P, S], BF16, kind="ExternalInput")
    cos4_d = nc.dram_tensor("cos4", [P, S], BF16, kind="ExternalInput")
    wq_d = nc.dram_tensor("wq", [HID, QD], F32R, kind="ExternalInput")
    wkv_d = nc.dram_tensor("wkv", [HID, 2 * HD], F32R, kind="ExternalInput")
    wo_d = nc.dram_tensor("wo", [QD, HID], BF16, kind="ExternalInput")
    wg_d = nc.dram_tensor("wg", [HID, INTER_LOC], BF16, kind="ExternalInput")
    wu_d = nc.dram_tensor("wu", [HID, INTER_LOC], BF16, kind="ExternalInput")
    wd_d = nc.dram_tensor("wd", [INTER_LOC, HID], BF16, kind="ExternalInput")
    ident_d = nc.dram_tensor("ident", [HD, HD], F32R, kind="ExternalInput")
    ones_d = nc.dram_tensor("ones", [P, 1], F32R, kind="ExternalInput")
    masks_d = nc.dram_tensor("masks", [P, 4 * Q], BF16, kind="ExternalInput")
    onesb_d = nc.dram_tensor("onesb", [P, 1], BF16, kind="ExternalInput")
    outT_d = nc.dram_tensor("outT", [HID, S], F16, kind="ExternalOutput")

    with tile.TileContext(nc) as tc, nc.allow_low_precision(
            reason="bf16/f16 activations within a 2e-2 rel-err budget"):
        with (
            tc.tile_pool(name="const", bufs=1) as const,
            tc.tile_pool(name="dramp", bufs=1, space="DRAM") as dram,
        ):
            ones1 = const.tile([P, 1], F32R)
            eps1 = const.tile([P, 1], F32)
            nc.gpsimd.memset(eps1, EPS)
            nc.sync.dma_start(ones1, ones_d[:, :])

            ar_in = [dram.tile([HID, Q], ARDT, name=f"ar_in{i}",
                               tag=f"ar_in{i}") for i in range(NQ)]
            ar_out = [dram.tile([HID, Q], ARDT, addr_space="Shared",
                                name=f"ar_out{i}", tag=f"ar_out{i}")
                      for i in range(NQ)]
            bc1_dram = dram.tile([8, XC], F32R)          # phase-1 rms rows
            bc2_dram = dram.tile([NQ, 2, 2, Q], F32R)    # softmax denoms
            bc4_dram = dram.tile([NQ, Q], F32R)          # rms2 rows

            # ======== persistent attention tensors =========================
            with tc.tile_pool(name="keep", bufs=1) as keep:
                sin4 = keep.tile([P, S], BF16)
                cos4 = keep.tile([P, S], BF16)
                ident = keep.tile([HD, HD], F32R)
                masks = keep.tile([P, 4, Q], BF16)
                nc.scalar.dma_start(sin4, sin4_d[:, :])
                nc.scalar.dma_start(cos4, cos4_d[:, :])
                nc.sync.dma_start(ident, ident_d[:, :])
                nc.sync.dma_start(
                    masks, masks_d[:, :].rearrange("p (t n) -> p t n", t=4))
                qT = [keep.tile([P, S], BF16, tag=f"qT{m}", name=f"qT{m}")
                      for m in range(2)]
                kTdup = keep.tile([P, S], BF16, tag="kTdup")
                v_ones = keep.tile([P, 16, HD + 1], BF16, tag="v_ones")
                nc.sync.dma_start(
                    v_ones[:, :, HD:HD + 1],
                    _bcast(bass.AP(tensor=onesb_d.tensor
                                   if hasattr(onesb_d, "tensor") else onesb_d,
                                   offset=0, ap=[[0, 1], [0, 16], [0, 1]]),
                           P))
                # MLP weights resident (wd) / streamed (wg, wu)
                wd_all = keep.tile([P, 8, HID], BF16, tag="wd_all")
                nc.scalar.dma_start(
                    wd_all, wd_d[:, :].rearrange("(t p) m -> p t m", p=P))
                wo_all = keep.tile([P, 2, HID], BF16, tag="wo_all")
                nc.scalar.dma_start(
                    wo_all, wo_d[:, :].rearrange("(t p) m -> p t m", p=P))

                # ---- Phase 1: QKV + concurrent RMS1 stats + RoPE, per 256-
                #      token chunk.  QKV consumes raw x; inv_rms is applied
                #      at eviction through cos_eff/sin_eff.               ----
                with (
                    tc.tile_pool(name="p1w", bufs=1) as p1w,
                    tc.tile_pool(name="p1x", bufs=2) as p1x,
                    tc.tile_pool(name="p1s", bufs=1) as p1s,
                    tc.tile_pool(name="p1ps", bufs=2, space="PSUM") as p1ps,
                    tc.tile_pool(name="p1pss", bufs=2, space="PSUM") as p1pss,
                ):
                    wq_all = p1w.tile([P, 16, QD], F32R)
                    wkv_all = p1w.tile([P, 16, 2 * HD], F32R)
                    nc.scalar.dma_start(
                        wq_all, wq_d[:, :].rearrange("(t p) m -> p t m", p=P))
                    nc.scalar.dma_start(
                        wkv_all, wkv_d[:, :].rearrange("(t p) m -> p t m", p=P))

                    for xc in range(S // XC):
                        cc = slice(XC * xc, XC * (xc + 1))
                        xq = p1x.tile([P, 16, XC], F32R, tag="xq")
                        for t4 in range(4):
                            nc.sync.dma_start(
                                xq[:, 4 * t4:4 * (t4 + 1), :],
                                hT_d[512 * t4:512 * (t4 + 1), cc].rearrange(
                                    "(t p) m -> p t m", p=P))
                        q_ps = p1ps.tile([P, 2 * XC], F32, tag="qps")
                        kv_ps = p1ps.tile([P, XC], F32, tag="kvps")
                        ssq = p1pss.tile([1, XC], F32, tag="ssq")
                        for kt in range(16):
                            st, sp = (kt == 0), (kt == 15)
                            xt = xq[:, kt, :]
                            sq = p1s.tile([P, XC], F32R, tag="sq", bufs=3)
                            nc.scalar.activation(sq, xt, AF.Square)
                            for m in range(2):
                                nc.tensor.matmul(
                                    q_ps[:, XC * m:XC * (m + 1)],
                                    wq_all[:, kt, P * m:P * (m + 1)],
                                    xt, start=st, stop=sp)
                            nc.tensor.matmul(kv_ps, wkv_all[:, kt, :],
                                             xt, start=st, stop=sp)
                            nc.tensor.matmul(ssq, ones1, sq,
                                             start=st, stop=sp)
                        # rms chain on broadcast tiles (128 DVE lanes)
                        rms = p1s.tile([1, XC], F32R, tag="rms", bufs=2)
                        nc.scalar.activation(rms, ssq, AF.Sqrt,
                                             bias=eps1[0:1, :], scale=1.0 / HID)
                        nc.sync.dma_start(bc1_dram[xc:xc + 1, :], rms)
                        rmsb = p1s.tile([P, XC], F32R, tag="rmsb", bufs=2)
                        nc.sync.dma_start(rmsb, _bcast(bc1_dram[xc:xc + 1, :], P))
                        invb = p1s.tile([P, XC], F32R, tag="invb", bufs=2)
                        nc.vector.reciprocal(invb, rmsb)
                        cos_e = p1s.tile([P, XC], BF16, tag="cos_e", bufs=2)
                        sin_e = p1s.tile([P, XC], BF16, tag="sin_e", bufs=2)
                        nc.vector.tensor_mul(cos_e, cos4[:, cc], invb)
                        nc.vector.tensor_mul(sin_e, sin4[:, cc], invb)

                        # RoPE eviction: out = ps*cos_e + swap_half(ps)*sin_e
                        for m in range(2):
                            qp = q_ps[:, XC * m:XC * (m + 1)]
                            s1 = p1s.tile([P, XC], BF16, tag="s1", bufs=2)
                            s2 = p1s.tile([P, XC], BF16, tag="s2", bufs=2)
                            nc.vector.tensor_mul(s1, qp, cos_e)
                            for b in range(2):
                                x0 = slice(64 * b, 64 * b + 32)
                                x1s = slice(64 * b + 32, 64 * b + 64)
                                nc.vector.tensor_mul(
                                    s2[x0, :], qp[x1s, :], sin_e[x1s, :])
                                nc.vector.tensor_mul(
                                    s2[x1s, :], qp[x0, :], sin_e[x0, :])
                            nc.vector.tensor_add(qT[m][:, cc], s1, s2)
                        # k (rows 0:64 of kv), duplicated into 64:128
                        s1 = p1s.tile([64, XC], BF16, tag="s1k", bufs=2)
                        s2 = p1s.tile([64, XC], BF16, tag="s2k", bufs=2)
                        nc.vector.tensor_mul(s1, kv_ps[0:64, :], cos_e[0:64, :])
                        nc.vector.tensor_mul(
                            s2[0:32, :], kv_ps[32:64, :], sin_e[32:64, :])
                        nc.vector.tensor_mul(
                            s2[32:64, :], kv_ps[0:32, :], sin_e[0:32, :])
                        nc.vector.tensor_add(kTdup[0:64, cc], s1, s2)
                        nc.vector.tensor_copy(kTdup[64:128, cc],
                                              kTdup[0:64, cc])
                        # v: scale by inv, then PE-transpose into v_ones
                        vt = p1s.tile([64, XC], F32R, tag="vt", bufs=2)
                        nc.vector.tensor_mul(vt, kv_ps[64:128, :],
                                             invb[0:64, :])
                        for j in range(XC // P):
                            ktg = (XC // P) * xc + j
                            vtp = p1pss.tile([P, HD], F32R, tag="vtp")
                            nc.tensor.transpose(
                                vtp, vt[:, P * j:P * (j + 1)], ident)
                            nc.vector.tensor_copy(v_ones[:, ktg, 0:HD], vtp)

                # ---- Phases 2-4 interleaved: per quarter qc4 attention +
                #      o-proj + AllReduce; MLP chunk c emitted between later
                #      quarters so collectives hide behind PE work.        ----
                with (
                    tc.tile_pool(name="ps", bufs=1, space="PSUM") as psp,
                    tc.tile_pool(name="att", bufs=2) as att,
                    tc.tile_pool(name="mlp", bufs=2) as mlp,
                    tc.tile_pool(name="sc1", bufs=1) as sc1,
                ):
                    ps = [psp.tile([P, Q], F32, tag=f"ps{i}", name=f"ps{i}")
                          for i in range(8)]

                    def attn_quarter(qc4):
                        qs = slice(Q * qc4, Q * (qc4 + 1))
                        atn = []
                        for m in range(2):
                            pv = [ps[2 + 2 * (m % 2)][0:HD + 1, :],
                                  ps[3 + 2 * (m % 2)][0:HD + 1, :]]
                            nkt = 4 * qc4 + 4
                            for kt in range(nkt):
                                st, sp = (kt == 0), (kt == nkt - 1)
                                for b in range(2):
                                    rows = slice(64 * b, 64 * (b + 1))
                                    sc = ps[b][:, :]
                                    nc.tensor.matmul(
                                        sc,
                                        kTdup[rows, P * kt:P * (kt + 1)],
                                        qT[m][rows, qs],
                                        start=True, stop=True)
                                    pr = att.tile([P, Q], BF16, tag=f"pr{b}",
                                                  bufs=3)
                                    nc.scalar.activation(
                                        pr, sc, AF.Exp, scale=float(SCALE))
                                    if kt >= 4 * qc4:
                                        nc.vector.tensor_mul(
                                            pr, pr,
                                            masks[:, kt - 4 * qc4, :])
                                    nc.tensor.matmul(
                                        pv[b], v_ones[:, kt, :], pr,
                                        start=st, stop=sp)
                            at = att.tile([P, Q], BF16, tag=f"atn{m}")
                            atn.append(at)
                            for b in range(2):
                                # denom -> broadcast -> reciprocal (64 lanes)
                                den = att.tile([1, Q], F32R, tag=f"den{b}")
                                nc.scalar.copy(den, pv[b][HD:HD + 1, :])
                                slot = bc2_dram[qc4:qc4 + 1, m, b, :]
                                nc.sync.dma_start(slot, den)
                                recb = att.tile([64, Q], F32R, tag=f"recb{b}")
                                nc.sync.dma_start(recb, _bcast(slot, 64))
                                rec = att.tile([64, Q], F32R, tag=f"rec{b}")
                                nc.vector.reciprocal(rec, recb)
                                nc.vector.tensor_mul(
                                    at[64 * b:64 * (b + 1), :],
                                    pv[b][0:HD, :], rec)
                        # o-proj -> ar_in[qc4] (fp16)
                        for hm in range(16):
                            ops = ps[6 + (hm % 2)][:, :]
                            for kt2 in range(2):
                                nc.tensor.matmul(
                                    ops,
                                    wo_all[:, kt2, P * hm:P * (hm + 1)],
                                    atn[kt2],
                                    start=(kt2 == 0), stop=(kt2 == 1))
                            osb = att.tile([P, Q], ARDT, tag="osb", bufs=3)
                            nc.scalar.copy(osb, ops)
                            nc.gpsimd.dma_start(
                                ar_in[qc4][P * hm:P * (hm + 1), :], osb)
                        nc.gpsimd.collective_compute(
                            "AllReduce", ALU.add,
                            replica_groups=[list(range(N_CORES))],
                            ins=[ar_in[qc4][:, :].opt()],
                            outs=[ar_out[qc4][:, :].opt()])

                    def mlp_pre(c):
                        """x1 = x + attn (bf16), rms2 stats -> invb, xn2."""
                        cs = slice(Q * c, Q * (c + 1))
                        x1 = mlp.tile([P, 16, Q], BF16, tag="x1")
                        xn2 = mlp.tile([P, 16, Q], BF16, tag="xn2", bufs=1)
                        ssq2 = ps[4][0:1, :]
                        for kt in range(16):
                            rs = slice(P * kt, P * (kt + 1))
                            th = sc1.tile([P, Q], BF16, tag="th", bufs=3)
                            ta = sc1.tile([P, Q], ARDT, tag="ta", bufs=3)
                            nc.sync.dma_start(th, hTb_d[rs, cs])
                            nc.gpsimd.dma_start(ta, ar_out[c][rs, :])
                            nc.vector.tensor_add(x1[:, kt, :], th, ta)
                            sq = sc1.tile([P, Q], F32R, tag="sq2", bufs=3)
                            nc.scalar.activation(sq, x1[:, kt, :], AF.Square)
                            nc.tensor.matmul(ssq2, ones1, sq,
                                             start=(kt == 0), stop=(kt == 15))
                        rms = sc1.tile([1, Q], F32R, tag="rms2", bufs=2)
                        nc.scalar.activation(rms, ssq2, AF.Sqrt,
                                             bias=eps1[0:1, :], scale=1.0 / HID)
                        nc.sync.dma_start(bc4_dram[c:c + 1, :], rms)
                        rmsb = sc1.tile([P, Q], F32R, tag="rmsb2", bufs=2)
                        nc.sync.dma_start(rmsb, _bcast(bc4_dram[c:c + 1, :], P))
                        invb = sc1.tile([P, Q], F32R, tag="invb2", bufs=2)
                        nc.vector.reciprocal(invb, rmsb)
                        for kt in range(16):
                            nc.vector.tensor_mul(xn2[:, kt, :], x1[:, kt, :],
                                                 invb)
                        return x1, xn2

                    def mlp_main(c, x1, xn2):
                        cs = slice(Q * c, Q * (c + 1))
                        hmlp = mlp.tile([P, 8, Q], BF16, tag="hmlp", bufs=1)
                        for iq in range(8):
                            wg_t = mlp.tile([P, 16, P], BF16, tag="wgt",
                                            bufs=2)
                            wu_t = mlp.tile([P, 16, P], BF16, tag="wut",
                                            bufs=2)
                            nc.scalar.dma_start(
                                wg_t, wg_d[:, P * iq:P * (iq + 1)].rearrange(
                                    "(t p) m -> p t m", p=P))
                            nc.scalar.dma_start(
                                wu_t, wu_d[:, P * iq:P * (iq + 1)].rearrange(
                                    "(t p) m -> p t m", p=P))
                            gps = ps[0][:, :]
                            ups = ps[1][:, :]
                            for kt in range(16):
                                st, sp = (kt == 0), (kt == 15)
                                nc.tensor.matmul(gps, wg_t[:, kt, :],
                                                 xn2[:, kt, :],
                                                 start=st, stop=sp)
                                nc.tensor.matmul(ups, wu_t[:, kt, :],
                                                 xn2[:, kt, :],
                                                 start=st, stop=sp)
                            sg = sc1.tile([P, Q], BF16, tag="sg", bufs=2)
                            nc.scalar.activation(sg, gps, AF.Silu)
                            nc.vector.tensor_mul(hmlp[:, iq, :], sg, ups)
                        for hm in range(16):
                            dps = ps[2 + (hm % 2)][:, :]
                            for kt8 in range(8):
                                nc.tensor.matmul(dps, wd_all[:, kt8,
                                                             P * hm:P * (hm + 1)],
                                                 hmlp[:, kt8, :],
                                                 start=(kt8 == 0),
                                                 stop=(kt8 == 7))
                            dsb = sc1.tile([P, Q], F16, tag="dsb", bufs=3)
                            nc.vector.scalar_tensor_tensor(
                                dsb, x1[:, hm, :], 1.0 / N_CORES, dps,
                                op0=ALU.mult, op1=ALU.add)
                            nc.gpsimd.dma_start(outT_d[P * hm:P * (hm + 1), cs],
                                                dsb)

                    # ---- interleaved emission schedule ----
                    attn_quarter(0)
                    attn_quarter(1)
                    attn_quarter(2)
                    st0 = mlp_pre(0)
                    mlp_main(0, *st0)
                    attn_quarter(3)
                    st1 = mlp_pre(1)
                    mlp_main(1, *st1)
                    st2 = mlp_pre(2)
                    mlp_main(2, *st2)
                    st3 = mlp_pre(3)
                    mlp_main(3, *st3)

    nc.compile()
    return nc


_CACHE = {}


def _get_nc():
    if "nc" not in _CACHE:
        _CACHE["nc"] = build()
    return _CACHE["nc"]


def _prep_inputs(inputs):
    """Shard + preprocess full inputs into 8 per-core in_maps."""
    f = lambda k: np.asarray(inputs[k], dtype=np.float32)
    hidden = f("hidden_states")[0]                 # [S, HID]
    sin_t, cos_t = f("sin_table"), f("cos_table")  # [S, 32]
    ln1, ln2 = f("ln1_w"), f("ln2_w")
    wq = f("wq") * ln1[:, None]
    wk = f("wk") * ln1[:, None]
    wv = f("wv") * ln1[:, None]
    wo = f("wo").astype(ml_dtypes.bfloat16)
    wg = (f("w_gate") * ln2[:, None]).astype(ml_dtypes.bfloat16)
    wu = (f("w_up") * ln2[:, None]).astype(ml_dtypes.bfloat16)
    wd = f("w_down").astype(ml_dtypes.bfloat16)

    hT = np.ascontiguousarray(hidden.T)
    hTb = hT.astype(ml_dtypes.bfloat16)
    # rows per 64-block: [+sinT (x0 source); -sinT (x1 source)]
    sin4 = np.ascontiguousarray(
        np.tile(np.concatenate([sin_t.T, -sin_t.T], axis=0),
                (2, 1))).astype(ml_dtypes.bfloat16)
    cos4 = np.ascontiguousarray(np.tile(cos_t.T, (4, 1))).astype(
        ml_dtypes.bfloat16)
    ident = np.eye(HD, dtype=np.float32)
    ones = np.ones((P, 1), dtype=np.float32)
    onesb = np.ones((P, 1), dtype=ml_dtypes.bfloat16)
    rr = np.arange(P)[:, None]
    cols = np.arange(Q)[None, :]
    masks = np.concatenate(
        [(rr + 128 * t <= cols).astype(np.float32) for t in range(4)],
        axis=1).astype(ml_dtypes.bfloat16)

    in_maps = []
    for c in range(N_CORES):
        qs = slice(QD * c, QD * (c + 1))
        ks = slice(HD * c, HD * (c + 1))
        isl = slice(INTER_LOC * c, INTER_LOC * (c + 1))
        in_maps.append({
            "hT": hT,
            "hTb": hTb,
            "sin4": sin4,
            "cos4": cos4,
            "wq": np.ascontiguousarray(wq[:, qs]),
            "wkv": np.ascontiguousarray(
                np.concatenate([wk[:, ks], wv[:, ks]], axis=1)),
            "wo": np.ascontiguousarray(wo[qs, :]),
            "wg": np.ascontiguousarray(wg[:, isl]),
            "wu": np.ascontiguousarray(wu[:, isl]),
            "wd": np.ascontiguousarray(wd[isl, :]),
            "ident": ident,
            "ones": ones,
            "onesb": onesb,
            "masks": masks,
        })
    return in_maps


def kernel(**inputs):
    nc = _get_nc()
    in_maps = _prep_inputs(inputs)
    res = run_bass_kernel_spmd(nc, in_maps, core_ids=list(range(N_CORES)))
    acc = np.zeros((HID, S), dtype=np.float32)
    for c in range(N_CORES):
        acc += res.results[c]["outT"].astype(np.float32)
    return np.ascontiguousarray(acc.T)[None, :, :]


# revision 16
# speedup vs baseline: 1.2033x; 1.0838x over previous
"""Trainium2 Bass kernel for nn_DecoderLayer_66408784331382 (v3).

Single transformer decoder layer (RMSNorm + GQA attention w/ RoPE + RMSNorm +
SwiGLU MLP), tensor-parallel over 8 NeuronCores:

  - per core: 4 of 32 Q heads, 1 of 8 KV heads, 1024 of 8192 MLP inter cols,
    matching row-shards of wo / w_down.
  - activations transposed on device ([hid, tok]); host supplies x.T in fp16.
  - RMS1 is folded *after* the QKV projection ((x*inv)@W == (x@W)*inv): QKV
    matmuls run immediately on raw x while sum-of-squares stats compute
    concurrently; inv_rms folds into the RoPE cos/sin multipliers.
  - attention path is fp16 (same PE rate, half the DVE/ACT/DMA cost of fp32);
    MLP is bf16; all PSUM accumulation fp32.  Softmax exp carries a -2.0 bias
    (cancels in the normalization) to keep fp16 headroom.
  - reciprocals use the approximate custom-DVE op on [128,N]/[64,N] fp32
    broadcast tiles (DRAM round-trip with partition-stride-0 APs).
  - the attention-output AllReduce is split into four 512-token fp16 chunks;
    emission is interleaved via generators: gate/up matmuls of MLP chunk c
    fill the PE while attention of later quarters waits on the exp chain, and
    down-proj of chunk c overlaps gate/up of chunk c+1.  Activation-table
    switches (Exp/Sqrt/Silu) are kept to ~2 per chunk.
  - DMA queues: sync = streaming loads (x, gate/up weights), scalar = weight
    preamble, gpsimd = gated traffic (collectives, broadcasts, outputs).
  - down-proj partials (+ x1/8 residual) are written fp16 and summed on host.

kernel(**inputs) takes the FULL fp32 inputs of reference.setup_inputs() and
returns the FULL [1, 2048, 2048] fp32 output.
"""

import sys

if "/opt/trn_rl_repo" not in sys.path:
    sys.path.insert(0, "/opt/trn_rl_repo")

import numpy as np
import ml_dtypes

import concourse.bass as bass
import concourse.mybir as mybir
import concourse.tile as tile
from concourse import bacc
from concourse.bass_utils import run_bass_kernel_spmd

# ---- problem constants (hardcoded per contract) ----
N_CORES = 8
S = 2048
HID = 2048
HD = 64
NH = 32
INTER = 8192
EPS = 1e-6

QD = (NH // N_CORES) * HD        # 256 local q cols (2 tiles of 128)
INTER_LOC = INTER // N_CORES     # 1024
SCALE = 1.0 / np.sqrt(HD)
EXPB = -2.0                      # softmax exp bias (cancels in normalization)

F32 = mybir.dt.float32
F32R = mybir.dt.float32r
BF16 = mybir.dt.bfloat16
F16 = mybir.dt.float16

P = 128
Q = 512      # token quarter (phase-1 chunk, attention block, MLP chunk)
NQ = S // Q  # 4
ARDT = F16   # collective dtype
AF = mybir.ActivationFunctionType
ALU = mybir.AluOpType


def _bcast(ap, parts):
    """View a [1, N] AP as [parts, N] via partition-stride-0 (DMA broadcast)."""
    return bass.AP(tensor=ap.tensor, offset=ap.offset,
                   ap=[[0, parts]] + [list(p) for p in ap.ap[1:]])


def _drive(*gens):
    """Round-robin the emission generators until all are exhausted."""
    active = [g for g in gens if g is not None]
    while active:
        for g in list(active):
            try:
                next(g)
            except StopIteration:
                active.remove(g)


def build():
    nc = bacc.Bacc("TRN2", target_bir_lowering=False, debug=False,
                   num_devices=N_CORES)

    hTb_d = nc.dram_tensor("hTb", [HID, S], F16, kind="ExternalInput")
    sin4_d = nc.dram_tensor("sin4", [P, S], F16, kind="ExternalInput")
    cos4_d = nc.dram_tensor("cos4", [P, S], F16, kind="ExternalInput")
    wq_d = nc.dram_tensor("wq", [HID, QD], F16, kind="ExternalInput")
    wkv_d = nc.dram_tensor("wkv", [HID, 2 * HD], F16, kind="ExternalInput")
    wo_d = nc.dram_tensor("wo", [QD, HID], F16, kind="ExternalInput")
    wg_d = nc.dram_tensor("wg", [HID, INTER_LOC], BF16, kind="ExternalInput")
    wu_d = nc.dram_tensor("wu", [HID, INTER_LOC], BF16, kind="ExternalInput")
    wd_d = nc.dram_tensor("wd", [INTER_LOC, HID], BF16, kind="ExternalInput")
    ident_d = nc.dram_tensor("ident", [HD, HD], F32R, kind="ExternalInput")
    ones_d = nc.dram_tensor("ones", [P, 1], F32R, kind="ExternalInput")
    masks_d = nc.dram_tensor("masks", [P, 4 * Q], F16, kind="ExternalInput")
    onesh_d = nc.dram_tensor("onesh", [P, 1], F16, kind="ExternalInput")
    outT_d = nc.dram_tensor("outT", [HID, S], F16, kind="ExternalOutput")

    with tile.TileContext(nc) as tc, nc.allow_low_precision(
            reason="f16/bf16 activations within a 2e-2 rel-err budget"):
        with (
            tc.tile_pool(name="const", bufs=1) as const,
            tc.tile_pool(name="dramp", bufs=1, space="DRAM") as dram,
        ):
            ones1 = const.tile([P, 1], F32R)
            eps1 = const.tile([P, 1], F32)
            expb1 = const.tile([P, 1], F32)
            nc.gpsimd.memset(eps1, EPS)
            nc.gpsimd.memset(expb1, EXPB)
            nc.gpsimd.dma_start(ones1, ones_d[:, :])

            ar_in = [dram.tile([HID, Q], ARDT, name=f"ar_in{i}",
                               tag=f"ar_in{i}") for i in range(NQ)]
            ar_out = [dram.tile([HID, Q], ARDT, addr_space="Shared",
                                name=f"ar_out{i}", tag=f"ar_out{i}")
                      for i in range(NQ)]
            bc1_dram = dram.tile([NQ, Q], F32)           # phase-1 rms rows
            bc2_dram = dram.tile([NQ, 2, 2, Q], F32)     # softmax denoms
            bc4_dram = dram.tile([NQ, Q], F32)           # rms2 rows

            # ======== persistent tensors ===================================
            with tc.tile_pool(name="keep", bufs=1) as keep:
                ident = keep.tile([HD, HD], F32R)
                masks = keep.tile([P, 4, Q], F16)
                nc.gpsimd.dma_start(ident, ident_d[:, :])
                nc.gpsimd.dma_start(
                    masks, masks_d[:, :].rearrange("p (t n) -> p t n", t=4))
                qT = [keep.tile([P, S], F16, tag=f"qT{m}", name=f"qT{m}")
                      for m in range(2)]
                kTdup = keep.tile([P, S], F16, tag="kTdup")
                v_ones = keep.tile([P, 16, HD + 1], F16, tag="v_ones")
                nc.gpsimd.dma_start(
                    v_ones[:, :, HD:HD + 1],
                    _bcast(bass.AP(tensor=onesh_d.tensor
                                   if hasattr(onesh_d, "tensor") else onesh_d,
                                   offset=0, ap=[[0, 1], [0, 16], [0, 1]]),
                           P))
                wo_all = keep.tile([P, 2, HID], F16, tag="wo_all")
                wd_all = keep.tile([P, 8, HID], BF16, tag="wd_all")

                # ---- Phase 1: QKV + concurrent RMS1 stats + RoPE ----------
                with (
                    tc.tile_pool(name="p1w", bufs=1) as p1w,
                    tc.tile_pool(name="p1x", bufs=2) as p1x,
                    tc.tile_pool(name="p1s", bufs=1) as p1s,
                    tc.tile_pool(name="p1ps", bufs=2, space="PSUM") as p1ps,
                    tc.tile_pool(name="p1pss", bufs=1, space="PSUM") as p1pss,
                ):
                    wq_all = p1w.tile([P, 16, QD], F16)
                    wkv_all = p1w.tile([P, 16, 2 * HD], F16)
                    nc.scalar.dma_start(
                        wq_all, wq_d[:, :].rearrange("(t p) m -> p t m", p=P))
                    nc.scalar.dma_start(
                        wkv_all, wkv_d[:, :].rearrange("(t p) m -> p t m", p=P))
                    # wo/wd after the phase-1 weights on the scalar queue
                    nc.scalar.dma_start(
                        wo_all, wo_d[:, :].rearrange("(t p) m -> p t m", p=P))
                    nc.scalar.dma_start(
                        wd_all, wd_d[:, :].rearrange("(t p) m -> p t m", p=P))
                    sin4 = p1w.tile([P, S], F16)
                    cos4 = p1w.tile([P, S], F16)
                    nc.gpsimd.dma_start(sin4, sin4_d[:, :])
                    nc.gpsimd.dma_start(cos4, cos4_d[:, :])

                    for xc in range(NQ):
                        cc = slice(Q * xc, Q * (xc + 1))
                        xq = p1x.tile([P, 16, Q], F16, tag="xq")
                        for t4 in range(4):
                            nc.sync.dma_start(
                                xq[:, 4 * t4:4 * (t4 + 1), :],
                                hTb_d[512 * t4:512 * (t4 + 1), cc].rearrange(
                                    "(t p) m -> p t m", p=P))
                        q_ps = [p1ps.tile([P, Q], F32, tag=f"qm{m}",
                                          name=f"qm{m}") for m in range(2)]
                        kv_ps = p1ps.tile([P, Q], F32, tag="kvps")
                        ssq = p1pss.tile([1, Q], F32, tag="ssq")
                        for kt in range(16):
                            st, sp = (kt == 0), (kt == 15)
                            xt = xq[:, kt, :]
                            sq = p1s.tile([P, Q], F32R, tag="sq", bufs=3)
                            nc.scalar.activation(sq, xt, AF.Square)
                            for m in range(2):
                                nc.tensor.matmul(
                                    q_ps[m],
                                    wq_all[:, kt, P * m:P * (m + 1)],
                                    xt, start=st, stop=sp)
                            nc.tensor.matmul(kv_ps, wkv_all[:, kt, :],
                                             xt, start=st, stop=sp)
                            nc.tensor.matmul(ssq, ones1, sq,
                                             start=st, stop=sp)
                        # rms chain: Sqrt -> broadcast -> approx reciprocal
                        rms = p1s.tile([1, Q], F32, tag="rms", bufs=2)
                        nc.scalar.activation(rms, ssq, AF.Sqrt,
                                             bias=eps1[0:1, :], scale=1.0 / HID)
                        nc.gpsimd.dma_start(bc1_dram[xc:xc + 1, :], rms)
                        rmsb = p1s.tile([P, Q], F32, tag="rmsb", bufs=2)
                        nc.gpsimd.dma_start(rmsb,
                                            _bcast(bc1_dram[xc:xc + 1, :], P))
                        invb = p1s.tile([P, Q], F32, tag="invb", bufs=2)
                        nc.vector.reciprocal_approx_fast(invb, rmsb)
                        cos_e = p1s.tile([P, Q], F16, tag="cos_e", bufs=2)
                        sin_e = p1s.tile([P, Q], F16, tag="sin_e", bufs=2)
                        nc.vector.tensor_mul(cos_e, cos4[:, cc], invb)
                        nc.vector.tensor_mul(sin_e, sin4[:, cc], invb)

                        # RoPE eviction: out = ps*cos_e + swap_half(ps)*sin_e
                        for m in range(2):
                            qp = q_ps[m]
                            s1 = p1s.tile([P, Q], F16, tag="s1", bufs=2)
                            s2 = p1s.tile([P, Q], F16, tag="s2", bufs=2)
                            nc.vector.tensor_mul(s1, qp, cos_e)
                            for b in range(2):
                                x0 = slice(64 * b, 64 * b + 32)
                                x1s = slice(64 * b + 32, 64 * b + 64)
                                nc.vector.tensor_mul(
                                    s2[x0, :], qp[x1s, :], sin_e[x1s, :])
                                nc.vector.tensor_mul(
                                    s2[x1s, :], qp[x0, :], sin_e[x0, :])
                            nc.vector.tensor_add(qT[m][:, cc], s1, s2)
                        # k (rows 0:64 of kv), duplicated into 64:128
                        s1 = p1s.tile([64, Q], F16, tag="s1k", bufs=2)
                        s2 = p1s.tile([64, Q], F16, tag="s2k", bufs=2)
                        nc.vector.tensor_mul(s1, kv_ps[0:64, :],
                                             cos_e[0:64, :])
                        nc.vector.tensor_mul(
                            s2[0:32, :], kv_ps[32:64, :], sin_e[32:64, :])
                        nc.vector.tensor_mul(
                            s2[32:64, :], kv_ps[0:32, :], sin_e[0:32, :])
                        nc.vector.tensor_add(kTdup[0:64, cc], s1, s2)
                        nc.vector.tensor_copy(kTdup[64:128, cc],
                                              kTdup[0:64, cc])
                        # v: scale by inv, then PE-transpose into v_ones
                        vt = p1s.tile([64, Q], F32R, tag="vt", bufs=2)
                        nc.vector.tensor_mul(vt, kv_ps[64:128, :],
                                             invb[0:64, :])
                        for j in range(Q // P):
                            ktg = (Q // P) * xc + j
                            vtp = p1pss.tile([P, HD], F32R, tag="vtp")
                            nc.tensor.transpose(
                                vtp, vt[:, P * j:P * (j + 1)], ident)
                            nc.vector.tensor_copy(v_ones[:, ktg, 0:HD], vtp)

                # ---- Phases 2-4, interleaved via emission generators ------
                with (
                    tc.tile_pool(name="ps", bufs=1, space="PSUM") as psp,
                    tc.tile_pool(name="att", bufs=2) as att,
                    tc.tile_pool(name="mlp", bufs=2) as mlp,
                    tc.tile_pool(name="sc1", bufs=1) as sc1,
                ):
                    ps = [psp.tile([P, Q], F32, tag=f"ps{i}", name=f"ps{i}")
                          for i in range(8)]

                    def g_attn(qc4):
                        """Scores/softmax/PV per m, then o-proj, then AR."""
                        qs = slice(Q * qc4, Q * (qc4 + 1))
                        atn = []
                        for m in range(2):
                            pv = [ps[2 + 2 * m][0:HD + 1, :],
                                  ps[3 + 2 * m][0:HD + 1, :]]
                            nkt = 4 * qc4 + 4
                            for kt in range(nkt):
                                st, sp = (kt == 0), (kt == nkt - 1)
                                for b in range(2):
                                    rows = slice(64 * b, 64 * (b + 1))
                                    sc = ps[(2 * kt + b) % 2][:, :]
                                    nc.tensor.matmul(
                                        sc,
                                        kTdup[rows, P * kt:P * (kt + 1)],
                                        qT[m][rows, qs],
                                        start=True, stop=True)
                                    pr = att.tile([P, Q], F16, tag=f"pr{b}",
                                                  bufs=2)
                                    nc.scalar.activation(
                                        pr, sc, AF.Exp, bias=expb1,
                                        scale=float(SCALE))
                                    if kt >= 4 * qc4:
                                        nc.vector.tensor_mul(
                                            pr, pr,
                                            masks[:, kt - 4 * qc4, :])
                                    nc.tensor.matmul(
                                        pv[b], v_ones[:, kt, :], pr,
                                        start=st, stop=sp)
                                yield
                            at = att.tile([P, Q], F16, tag=f"atn{m}")
                            atn.append(at)
                            for b in range(2):
                                den = att.tile([1, Q], F32, tag=f"den{b}")
                                nc.scalar.copy(den, pv[b][HD:HD + 1, :])
                                slot = bc2_dram[qc4:qc4 + 1, m, b, :]
                                nc.gpsimd.dma_start(slot, den)
                                recb = att.tile([64, Q], F32, tag=f"recb{b}",
                                                bufs=1)
                                nc.gpsimd.dma_start(recb, _bcast(slot, 64))
                                rec = att.tile([64, Q], F32, tag=f"rec{b}",
                                               bufs=1)
                                nc.vector.reciprocal_approx_fast(rec, recb)
                                nc.vector.tensor_mul(
                                    at[64 * b:64 * (b + 1), :],
                                    pv[b][0:HD, :], rec)
                            yield
                        for hm in range(16):
                            ops = ps[hm % 2][:, :]
                            for kt2 in range(2):
                                nc.tensor.matmul(
                                    ops,
                                    wo_all[:, kt2, P * hm:P * (hm + 1)],
                                    atn[kt2],
                                    start=(kt2 == 0), stop=(kt2 == 1))
                            osb = att.tile([P, Q], ARDT, tag="osb", bufs=2)
                            nc.scalar.copy(osb, ops)
                            nc.gpsimd.dma_start(
                                ar_in[qc4][P * hm:P * (hm + 1), :], osb)
                            if hm % 2 == 1:
                                yield
                        nc.gpsimd.collective_compute(
                            "AllReduce", ALU.add,
                            replica_groups=[list(range(N_CORES))],
                            ins=[ar_in[qc4][:, :].opt()],
                            outs=[ar_out[qc4][:, :].opt()])

                    def g_pre(c):
                        """x1 = x + attn (f16), rms2 stats -> invb -> xn2."""
                        cs = slice(Q * c, Q * (c + 1))
                        x1 = mlp.tile([P, 16, Q], F16, tag="x1")
                        xn2 = mlp.tile([P, 16, Q], BF16, tag="xn2")
                        ssq2 = ps[4][0:1, :]
                        for kt in range(16):
                            rs = slice(P * kt, P * (kt + 1))
                            th = sc1.tile([P, Q], F16, tag="th", bufs=2)
                            ta = sc1.tile([P, Q], ARDT, tag="ta", bufs=2)
                            nc.sync.dma_start(th, hTb_d[rs, cs])
                            nc.gpsimd.dma_start(ta, ar_out[c][rs, :])
                            nc.vector.tensor_add(x1[:, kt, :], th, ta)
                            sq = sc1.tile([P, Q], F32R, tag="sq2", bufs=2)
                            nc.scalar.activation(sq, x1[:, kt, :], AF.Square)
                            nc.tensor.matmul(ssq2, ones1, sq,
                                             start=(kt == 0), stop=(kt == 15))
                            if kt % 4 == 3:
                                yield
                        rms = sc1.tile([1, Q], F32, tag="rms2", bufs=2)
                        nc.scalar.activation(rms, ssq2, AF.Sqrt,
                                             bias=eps1[0:1, :], scale=1.0 / HID)
                        nc.gpsimd.dma_start(bc4_dram[c:c + 1, :], rms)
                        rmsb = sc1.tile([P, Q], F32, tag="rmsb2", bufs=1)
                        nc.gpsimd.dma_start(rmsb,
                                            _bcast(bc4_dram[c:c + 1, :], P))
                        invb = sc1.tile([P, Q], F32, tag="invb2", bufs=1)
                        nc.vector.reciprocal_approx_fast(invb, rmsb)
                        for kt in range(16):
                            nc.vector.tensor_mul(xn2[:, kt, :], x1[:, kt, :],
                                                 invb)
                            if kt % 8 == 7:
                                yield
                        g_pre.out[c] = (x1, xn2)
                    g_pre.out = {}

                    def g_gu(c):
                        """gate/up matmuls for chunk c (PE + weight DMA only;
                        no scalar work, so it interleaves with exp safely)."""
                        x1, xn2 = g_pre.out[c]
                        for iq in range(8):
                            wg_t = mlp.tile([P, 16, P], BF16, tag="wgt",
                                            bufs=2)
                            wu_t = mlp.tile([P, 16, P], BF16, tag="wut",
                                            bufs=2)
                            nc.sync.dma_start(
                                wg_t, wg_d[:, P * iq:P * (iq + 1)].rearrange(
                                    "(t p) m -> p t m", p=P))
                            nc.sync.dma_start(
                                wu_t, wu_d[:, P * iq:P * (iq + 1)].rearrange(
                                    "(t p) m -> p t m", p=P))
                            gps = ps[6][:, :]
                            ups = ps[7][:, :]
                            for kt in range(16):
                                st, sp = (kt == 0), (kt == 15)
                                nc.tensor.matmul(gps, wg_t[:, kt, :],
                                                 xn2[:, kt, :],
                                                 start=st, stop=sp)
                                nc.tensor.matmul(ups, wu_t[:, kt, :],
                                                 xn2[:, kt, :],
                                                 start=st, stop=sp)
                                if kt % 2 == 1:
                                    yield
                            # evict raw gate/up on DVE (table-neutral);
                            # silu happens batched in g_fin.
                            graw = mlp.tile([P, Q], BF16, tag=f"graw{iq}",
                                            bufs=1)
                            nc.vector.tensor_copy(graw, gps)
                            upr = mlp.tile([P, Q], BF16, tag=f"upr{iq}",
                                           bufs=1)
                            nc.vector.tensor_copy(upr, ups)
                            g_gu.out[(c, iq)] = (graw, upr)
                            yield
                    g_gu.out = {}

                    def g_fin(c):
                        """silu + hmlp, then down-proj + residual + out DMA."""
                        x1, _ = g_pre.out[c]
                        hmlp = mlp.tile([P, 8, Q], BF16, tag="hmlp", bufs=1)
                        for iq in range(8):
                            graw, upr = g_gu.out.pop((c, iq))
                            sg = sc1.tile([P, Q], F16, tag="sg", bufs=1)
                            nc.scalar.activation(sg, graw, AF.Silu)
                            nc.vector.tensor_mul(hmlp[:, iq, :], sg, upr)
                            if iq % 4 == 3:
                                yield
                        for hm in range(16):
                            dps = ps[2 + (hm % 2)][:, :]
                            for kt8 in range(8):
                                nc.tensor.matmul(
                                    dps,
                                    wd_all[:, kt8, P * hm:P * (hm + 1)],
                                    hmlp[:, kt8, :],
                                    start=(kt8 == 0), stop=(kt8 == 7))
                            dsb = sc1.tile([P, Q], F16, tag="dsb", bufs=2)
                            nc.vector.scalar_tensor_tensor(
                                dsb, x1[:, hm, :], 1.0 / N_CORES, dps,
                                op0=ALU.mult, op1=ALU.add)
                            nc.gpsimd.dma_start(
                                outT_d[P * hm:P * (hm + 1),
                                       Q * c:Q * (c + 1)], dsb)
                            yield

                    # ---- interleaved emission schedule ----
                    _drive(g_attn(0))
                    _drive(g_attn(1))
                    _drive(g_attn(2))
                    _drive(g_pre(0))
                    _drive(g_attn(3), g_gu(0))
                    _drive(g_pre(1))
                    _drive(g_fin(0), g_gu(1))
                    _drive(g_pre(2))
                    _drive(g_fin(1), g_gu(2))
                    _drive(g_pre(3))
                    _drive(g_fin(2), g_gu(3))
                    _drive(g_fin(3))

    nc.compile()
    return nc


_CACHE = {}


def _get_nc():
    if "nc" not in _CACHE:
        _CACHE["nc"] = build()
    return _CACHE["nc"]


def _prep_inputs(inputs):
    """Shard + preprocess full inputs into 8 per-core in_maps."""
    f = lambda k: np.asarray(inputs[k], dtype=np.float32)
    hidden = f("hidden_states")[0]                 # [S, HID]
    sin_t, cos_t = f("sin_table"), f("cos_table")  # [S, 32]
    ln1, ln2 = f("ln1_w"), f("ln2_w")
    f16 = ml_dtypes.float16 if hasattr(ml_dtypes, "float16") else np.float16
    wq = (f("wq") * ln1[:, None]).astype(np.float16)
    wk = (f("wk") * ln1[:, None]).astype(np.float16)
    wv = (f("wv") * ln1[:, None]).astype(np.float16)
    wo = f("wo").astype(np.float16)
    wg = (f("w_gate") * ln2[:, None]).astype(ml_dtypes.bfloat16)
    wu = (f("w_up") * ln2[:, None]).astype(ml_dtypes.bfloat16)
    wd = f("w_down").astype(ml_dtypes.bfloat16)

    hTb = np.ascontiguousarray(hidden.T).astype(np.float16)
    # rows per 64-block: [+sinT (x0 source); -sinT (x1 source)]
    sin4 = np.ascontiguousarray(
        np.tile(np.concatenate([sin_t.T, -sin_t.T], axis=0),
                (2, 1))).astype(np.float16)
    cos4 = np.ascontiguousarray(np.tile(cos_t.T, (4, 1))).astype(np.float16)
    ident = np.eye(HD, dtype=np.float32)
    ones = np.ones((P, 1), dtype=np.float32)
    onesh = np.ones((P, 1), dtype=np.float16)
    rr = np.arange(P)[:, None]
    cols = np.arange(Q)[None, :]
    masks = np.concatenate(
        [(rr + 128 * t <= cols).astype(np.float32) for t in range(4)],
        axis=1).astype(np.float16)

    in_maps = []
    for c in range(N_CORES):
        qs = slice(QD * c, QD * (c + 1))
        ks = slice(HD * c, HD * (c + 1))
        isl = slice(INTER_LOC * c, INTER_LOC * (c + 1))
        in_maps.append({
            "hTb": hTb,
            "sin4": sin4,
            "cos4": cos4,
            "wq": np.ascontiguousarray(wq[:, qs]),
            "wkv": np.ascontiguousarray(
                np.concatenate([wk[:, ks], wv[:, ks]], axis=1)),
            "wo": np.ascontiguousarray(wo[qs, :]),
            "wg": np.ascontiguousarray(wg[:, isl]),
            "wu": np.ascontiguousarray(wu[:, isl]),
            "wd": np.ascontiguousarray(wd[isl, :]),
            "ident": ident,
            "ones": ones,
            "onesh": onesh,
            "masks": masks,
        })
    return in_maps


def kernel(**inputs):
    nc = _get_nc()
    in_maps = _prep_inputs(inputs)
    res = run_bass_kernel_spmd(nc, in_maps, core_ids=list(range(N_CORES)))
    acc = np.zeros((HID, S), dtype=np.float32)
    for c in range(N_CORES):
        acc += res.results[c]["outT"].astype(np.float32)
    return np.ascontiguousarray(acc.T)[None, :, :]
